# revision 1
# baseline (speedup 1.0000x reference)
"""Trainium2 Bass kernel for ConvFourierKANLayer.

Computes y = conv2d(cos(x*k), w0) + conv2d(sin(x*k), w1) + bias for
k = 1..10 (G=10 Fourier orders), 3x3 kernel, pad 1, C=64 -> O=128.

Strategy (8 NeuronCores, data-parallel over batch B=16 -> 2 per core):
  - Host pre-transposes fouriercoeffs into 90 lhsT tiles [K=128, O=128]
    where K = (g_parity, c) packs two Fourier orders per matmul, and the
    tile index t enumerates (branch, g_pair, kh, kw).
  - On-chip, x rows are expanded to cos/sin of k*x. The DVE has no fp
    mod, so the argument reduction uses the fp32 magic-number rounding
    trick (only add/sub/mult, all ISA-valid tensor_scalar ops):
        u  = x*(k/2pi) + 16        (positive)
        v  = (u + 2^23) - 2^23     (= round(u), fp32 round-to-nearest)
        w  = u - v                 (in [-0.5, 0.5])
        sin(k*x) = Sin(w * 2pi)    (ScalarE spline, valid on [-pi, pi])
    cos uses u_c = u + 0.25 (phase + pi/2) through the same pipeline.
  - Implicit GEMM: per 8-row output strip, accumulate 90 matmuls
    (branch x g_pair x 3x3 taps) of [K=128]x[O=128] @ [K=128, N=512]
    into one PSUM bank, with float32r (full-rate fp22) arithmetic.
"""

import numpy as np

import concourse.bass as bass
import concourse.mybir as mybir
import concourse.tile as tile
from concourse import bacc
from concourse.bass_utils import run_bass_kernel_spmd

N_CORES = 8
B, C, H, W = 16, 64, 64, 64
O = 128
G = 10
BS = B // N_CORES  # batches per core
HT = 32  # output rows per chunk (4 psum banks of 8 rows each)
NT = 2 * 5 * 9  # weight tiles: branch x g_pair x 3 x 3

PI = float(np.pi)
TWO_PI = float(2 * np.pi)
MAGIC = 8388608.0  # 2^23: fp32 round-to-nearest-integer magic constant

F32 = mybir.dt.float32
F32R = mybir.dt.float32r

_CACHE = {}


def _build_module(reps=1, mmdt="f32r", ht=HT):
    MMDT = {"f32r": F32R, "bf16": mybir.dt.bfloat16, "fp16": mybir.dt.float16}[mmdt]
    nb = ht // 8  # psum banks per chunk
    nc = bacc.Bacc("TRN2", target_bir_lowering=False)
    x_d = nc.dram_tensor("x", [BS, C, H, W], F32, kind="ExternalInput")
    w_d = nc.dram_tensor("w", [128, NT, 128], MMDT, kind="ExternalInput")
    kv_d = nc.dram_tensor("kvec", [128, 5], F32, kind="ExternalInput")
    bias_d = nc.dram_tensor("biasv", [128, 1], F32, kind="ExternalInput")
    y_d = nc.dram_tensor("y", [BS, O, H, W], F32, kind="ExternalOutput")

    mult = mybir.AluOpType.mult
    add = mybir.AluOpType.add
    sin_f = mybir.ActivationFunctionType.Sin

    with tile.TileContext(nc) as tc:
        with (
            tc.tile_pool(name="const", bufs=1) as cpool,
            tc.tile_pool(name="wpool", bufs=1) as wpool,
            tc.tile_pool(name="gen", bufs=2) as gen,
            tc.tile_pool(name="cspool", bufs=3) as cspool,
            tc.tile_pool(name="outp", bufs=3) as outp,
            tc.tile_pool(name="psum", bufs=2, space="PSUM") as psum,
        ):
            wt = wpool.tile([128, NT, 128], MMDT)
            for wi in range(0, NT, 15):
                nc.sync.dma_start(
                    wt[:, wi : wi + 15, :], w_d[:, wi : wi + 15, :]
                )
            kvt = cpool.tile([128, 5], F32)
            nc.sync.dma_start(kvt[:], kv_d[:])
            bt = cpool.tile([128, 1], F32)
            nc.sync.dma_start(bt[:], bias_d[:])
            quarter = cpool.tile([128, 1], F32)
            nc.vector.memset(quarter[:], 0.25)

            for rep in range(reps):
              for b in range(BS):
                for h0 in range(0, H, ht):
                    gr0, gr1 = max(0, h0 - 1), min(H, h0 + ht + 1)
                    l0 = gr0 - (h0 - 1)  # local row index of first real row
                    nrows = gr1 - gr0
                    rs = slice(l0, l0 + nrows)

                    xd = gen.tile([128, ht + 2, W], F32, tag="xdup")
                    nc.sync.dma_start(xd[0:64, rs, :], x_d[b, :, gr0:gr1, :])
                    nc.sync.dma_start(xd[64:128, rs, :], x_d[b, :, gr0:gr1, :])

                    pss = [
                        psum.tile([128, 8, 64], F32, tag=f"ps{bk}",
                                  name=f"ps{bk}_{rep}_{b}_{h0}")
                        for bk in range(nb)
                    ]

                    for j in range(5):
                        # u = x*(k/2pi) + 16 ; v = round(u) ; w = u - v
                        us = gen.tile([128, ht + 2, W], F32, tag="us")
                        nc.vector.tensor_scalar(
                            us[:, rs, :], xd[:, rs, :],
                            kvt[:, j : j + 1], 16.0, mult, add,
                        )
                        uc = gen.tile([128, ht + 2, W], F32, tag="uc")
                        nc.scalar.activation(
                            uc[:, rs, :], us[:, rs, :],
                            mybir.ActivationFunctionType.Identity,
                            bias=quarter[:],
                        )

                        st = cspool.tile([128, ht + 2, W + 2], MMDT, tag="ss")
                        ct = cspool.tile([128, ht + 2, W + 2], MMDT, tag="cs")
                        for u_t, z in ((us, st), (uc, ct)):
                            v_t = gen.tile([128, ht + 2, W], F32, tag="vt", bufs=1)
                            nc.vector.tensor_scalar_add(
                                v_t[:, rs, :], u_t[:, rs, :], MAGIC
                            )
                            nc.vector.tensor_scalar_sub(
                                v_t[:, rs, :], v_t[:, rs, :], MAGIC
                            )
                            w_t = gen.tile([128, ht + 2, W], F32, tag="wt")
                            nc.vector.tensor_sub(
                                w_t[:, rs, :], u_t[:, rs, :], v_t[:, rs, :]
                            )
                            # zero borders (uint32 bitcast: memset can't
                            # encode fp32r), then fill interior with Sin
                            if mmdt == "f32r":
                                u32 = mybir.dt.uint32
                                zb = lambda ap: ap.bitcast(u32)
                            else:
                                zb = lambda ap: ap
                            nc.gpsimd.memset(zb(z[:, :, 0:1]), 0)
                            nc.gpsimd.memset(zb(z[:, :, W + 1 : W + 2]), 0)
                            if l0 == 1:
                                nc.gpsimd.memset(zb(z[:, 0:1, :]), 0)
                            if gr1 == H:
                                nc.gpsimd.memset(
                                    zb(z[:, ht + 1 : ht + 2, :]), 0
                                )
                            nc.scalar.activation(
                                z[:, rs, 1 : W + 1], w_t[:, rs, :], sin_f,
                                scale=TWO_PI,
                            )

                        for br in range(2):
                            src = ct if br == 0 else st
                            for dh in range(3):
                                for dw in range(3):
                                    t_idx = ((br * 5 + j) * 3 + dh) * 3 + dw
                                    for bk in range(nb):
                                        nc.tensor.matmul(
                                            pss[bk][:],
                                            wt[:, t_idx, :],
                                            src[
                                                :,
                                                8 * bk + dh : 8 * bk + dh + 8,
                                                dw : dw + 64,
                                            ],
                                            start=(j == 0 and br == 0
                                                   and dh == 0 and dw == 0),
                                            stop=(j == 4 and br == 1
                                                  and dh == 2 and dw == 2),
                                        )

                    for bk in range(nb):
                        ob = outp.tile([128, 8, 64], F32, tag="ob")
                        nc.vector.tensor_scalar_add(ob[:], pss[bk][:], bt[:, 0:1])
                        nc.sync.dma_start(
                            y_d[b, :, h0 + 8 * bk : h0 + 8 * bk + 8, :], ob[:]
                        )
    nc.finalize()
    return nc


def _get_module(reps=1, mmdt="f32r", ht=HT):
    key = ("nc", reps, mmdt, ht)
    if key not in _CACHE:
        _CACHE[key] = _build_module(reps, mmdt, ht)
    return _CACHE[key]


def _np_mmdt(mmdt):
    import ml_dtypes
    return {"f32r": np.float32, "bf16": ml_dtypes.bfloat16,
            "fp16": np.float16}[mmdt]


def _host_weights(fc, mmdt="f32r"):
    # fc: (2, O, C, kH, kW, G) -> w[p=(gp*64+c), t=(br,j,kh,kw), o]
    W6 = np.transpose(fc, (0, 5, 3, 4, 2, 1))  # (br, g, kh, kw, c, o)
    W6 = W6.reshape(2, 5, 2, 3, 3, 64, 128)  # (br, j, gp, kh, kw, c, o)
    Wt = np.transpose(W6, (0, 1, 3, 4, 2, 5, 6))  # (br, j, kh, kw, gp, c, o)
    Wt = Wt.reshape(NT, 128, 128)
    return np.ascontiguousarray(
        np.transpose(Wt, (1, 0, 2)).astype(_np_mmdt(mmdt))
    )


def _host_kvec():
    kvec = np.zeros((128, 5), np.float32)
    for j in range(5):
        kvec[0:64, j] = (2 * j + 1) / TWO_PI
        kvec[64:128, j] = (2 * j + 2) / TWO_PI
    return kvec


def kernel(x, fouriercoeffs, bias):
    x = np.ascontiguousarray(np.asarray(x, dtype=np.float32))
    fc = np.asarray(fouriercoeffs, dtype=np.float32)
    w_host = _host_weights(fc)
    kvec = _host_kvec()
    biasv = np.ascontiguousarray(
        np.asarray(bias, dtype=np.float32).reshape(128, 1)
    )

    nc = _get_module()
    in_maps = [
        {"x": x[i * BS : (i + 1) * BS], "w": w_host, "kvec": kvec, "biasv": biasv}
        for i in range(N_CORES)
    ]
    res = run_bass_kernel_spmd(nc, in_maps, list(range(N_CORES))).results
    return np.concatenate([res[i]["y"] for i in range(N_CORES)], axis=0)



# revision 21
# speedup vs baseline: 1.9248x; 1.9248x over previous
"""Trainium2 Bass kernel for ConvFourierKANLayer.

Computes y = conv2d(cos(x*k), w0) + conv2d(sin(x*k), w1) + bias for
k = 1..10 (G=10 Fourier orders), 3x3 kernel, pad 1, C=64 -> O=128.

Strategy (8 NeuronCores, data-parallel over batch B=16 -> 2 per core):
  - F(2,3) Winograd along H: the kh tap dimension collapses into 4
    transformed planes per 2 output rows (2 m-values/output instead of
    3 taps), cutting streamed PE rows 1.5x. The dw taps stay spatial
    (free AP shifts). Weight transform G=[[1,0,0],[.5,.5,.5],
    [.5,-.5,.5],[0,0,1]] is folded into the host-side weight repack.
  - Matmuls run in bf16 (1 cyc/row, and enables DVE 2x_1P mode for the
    bf16 transform subtracts/adds, which fp32 TT does not get).
  - Trig gen per g-pair j (partitions = 2 g's x 64 c):
      u  = x*(k/2pi) + 16          (ACT Identity, per-partition scale)
      v  = (u + 2^23) - 2^23       (DVE fused tensor_scalar round)
      w  = u - v  in [-0.5, 0.5]   (DVE tensor_tensor)
      a  = |w|                     (ACT Abs)
      sin(kx) = Sin(2pi * w)       (ACT spline, arg in [-pi, pi])
      cos(kx) = Sin(pi/2 - 2pi*a)  (ACT spline, arg in [-pi/2, pi/2])
  - Per 16-row output chunk: 4 PSUM banks accumulate m0..m3 planes
    over (j, branch, dw) = 30 matmuls each of N=512 rows; inverse
    transform y_even = m0+m1+m2+bias, y_odd = m1-m2-m3+bias runs on
    ACT (PSUM->SBUF copies) + DVE scalar_tensor_tensor.
"""

import numpy as np

import concourse.bass as bass
import concourse.mybir as mybir
import concourse.tile as tile
from concourse import bacc
from concourse.bass_utils import run_bass_kernel_spmd

N_CORES = 8
B, C, H, W = 16, 64, 64, 64
O = 128
G = 10
BS = B // N_CORES  # batches per core
HT = 16            # output rows per chunk (4 psum banks of 8 row-pairs)
NT = 4 * 5 * 2 * 3  # weight tiles: i x j x branch x dw = 120

PI = float(np.pi)
TWO_PI = float(2 * np.pi)
HALF_PI = float(np.pi / 2)
MAGIC = 8388608.0  # 2^23: fp32 round-to-nearest-integer magic constant

F32 = mybir.dt.float32
BF16 = mybir.dt.bfloat16

_CACHE = {}


def _build_module(reps=1, ht=HT):
    nc = bacc.Bacc("TRN2", target_bir_lowering=False)
    x_d = nc.dram_tensor("x", [BS, C, H, W], F32, kind="ExternalInput")
    w_d = nc.dram_tensor("w", [128, NT, 128], BF16, kind="ExternalInput")
    kv_d = nc.dram_tensor("kvec", [128, 5], F32, kind="ExternalInput")
    bias_d = nc.dram_tensor("biasv", [128, 1], F32, kind="ExternalInput")
    y_d = nc.dram_tensor("y", [BS, O, H, W], F32, kind="ExternalOutput")

    add = mybir.AluOpType.add
    sub = mybir.AluOpType.subtract
    mult = mybir.AluOpType.mult
    sin_f = mybir.ActivationFunctionType.Sin
    abs_f = mybir.ActivationFunctionType.Abs
    id_f = mybir.ActivationFunctionType.Identity

    IR = ht + 2  # input rows per chunk (halo of 1 above/below)
    RP = ht // 2  # row pairs

    with tile.TileContext(nc) as tc:
        CS_BUFS = 3
        V_BUFS = 3
        with (
            tc.tile_pool(name="const", bufs=1) as cpool,
            tc.tile_pool(name="wpool", bufs=1) as wpool,
            tc.tile_pool(name="gen", bufs=3) as gen,
            tc.tile_pool(name="cspool", bufs=CS_BUFS) as cspool,
            tc.tile_pool(name="vpool", bufs=V_BUFS) as vpool,
            tc.tile_pool(name="inv", bufs=3) as inv,
            tc.tile_pool(name="outp", bufs=3) as outp,
            tc.tile_pool(name="psum", bufs=2, space="PSUM") as psum,
        ):
            wt = wpool.tile([128, NT, 128], BF16)
            for wi in range(0, NT, 20):
                nc.sync.dma_start(wt[:, wi : wi + 20, :], w_d[:, wi : wi + 20, :])
            kvt = cpool.tile([128, 5], F32)
            nc.sync.dma_start(kvt[:], kv_d[:])
            bt = cpool.tile([128, 1], F32)
            nc.sync.dma_start(bt[:], bias_d[:])
            b16 = cpool.tile([128, 1], F32)
            nc.vector.memset(b16[:], 16.0)
            bhpi = cpool.tile([128, 1], F32)
            nc.vector.memset(bhpi[:], HALF_PI)

            def emit_inverse(pend):
                # y_even = m0+m1+m2+b, y_odd = m1-m2-m3+b
                pb, pr0, ps = pend
                t2 = inv.tile([128, RP, 64], F32, tag="t2")
                nc.scalar.activation(t2[:], ps[2][:], id_f)
                t12 = inv.tile([128, RP, 64], F32, tag="t12")
                nc.vector.scalar_tensor_tensor(
                    t12[:], ps[1][:], bt[:, 0:1], t2[:], add, add
                )
                t12m = inv.tile([128, RP, 64], F32, tag="t12m")
                nc.vector.scalar_tensor_tensor(
                    t12m[:], ps[1][:], bt[:, 0:1], t2[:], add, sub
                )
                yb = outp.tile([128, ht, 64], F32, tag="yb")
                nc.vector.scalar_tensor_tensor(
                    yb[:, 0:ht:2, :], ps[0][:], 0.0, t12[:], add, add
                )
                nc.vector.scalar_tensor_tensor(
                    yb[:, 1:ht:2, :], ps[3][:], -1.0, t12m[:], mult, add
                )
                nc.sync.dma_start(y_d[pb, :, pr0 : pr0 + ht, :], yb[:])

            pending = None
            it = 0  # global (chunk, j) iteration counter for border init
            for rep in range(reps):
              for b in range(BS):
                for r0 in range(0, H, ht):
                    gr0, gr1 = max(0, r0 - 1), min(H, r0 + ht + 1)
                    l0 = gr0 - (r0 - 1)  # local row of first real row (0/1)
                    nrows = gr1 - gr0
                    rs = slice(l0, l0 + nrows)

                    xd = gen.tile([128, IR, W], F32, tag="xd")
                    if l0 == 1:
                        nc.gpsimd.memset(xd[:, 0:1, :], 0)
                    if gr1 == H and l0 + nrows < IR:
                        nc.gpsimd.memset(xd[:, IR - 1 : IR, :], 0)
                    nc.sync.dma_start(xd[0:64, rs, :], x_d[b, :, gr0:gr1, :])
                    nc.sync.dma_start(xd[64:128, rs, :], x_d[b, :, gr0:gr1, :])

                    pss = [
                        psum.tile([128, RP, 64], F32, tag=f"ps{i}",
                                  name=f"ps{i}_{rep}_{b}_{r0}")
                        for i in range(4)
                    ]

                    for j in range(5):
                        u = gen.tile([128, IR, W], F32, tag="u")
                        nc.gpsimd.tensor_scalar(
                            u[:], xd[:], kvt[:, j : j + 1], 16.0, mult, add
                        )
                        v = gen.tile([128, IR, W], F32, tag="v")
                        nc.vector.tensor_scalar(v[:], u[:], MAGIC, MAGIC, add, sub)
                        w = gen.tile([128, IR, W], F32, tag="w")
                        nc.vector.tensor_sub(w[:], u[:], v[:])
                        a = gen.tile([128, IR, W], F32, tag="a")
                        nc.scalar.activation(a[:], w[:], abs_f)

                        st = cspool.tile([128, IR, W + 4], BF16, tag="st")
                        ct = cspool.tile([128, IR, W + 4], BF16, tag="ct")
                        for z in (st, ct):
                            nc.gpsimd.memset(z[:, :, 0:2], 0)
                            nc.gpsimd.memset(z[:, :, W + 2 : W + 4], 0)
                        nc.scalar.activation(
                            st[:, :, 2 : W + 2], w[:], sin_f, scale=TWO_PI
                        )
                        nc.scalar.activation(
                            ct[:, :, 2 : W + 2], a[:], sin_f,
                            bias=bhpi[:], scale=-TWO_PI,
                        )
                        # cos of zero-padded halo rows must be 0, not 1
                        if l0 == 1:
                            nc.gpsimd.memset(ct[:, 0:1, :], 0)
                        if gr1 == H and l0 + nrows < IR:
                            nc.gpsimd.memset(ct[:, IR - 1 : IR, :], 0)

                        vc = vpool.tile([128, 4, RP, W + 4], BF16, tag="vc")
                        vs = vpool.tile([128, 4, RP, W + 4], BF16, tag="vs")
                        for src, vt in ((ct, vc), (st, vs)):
                            nc.gpsimd.memset(vt[:, :, :, 0:2], 0)
                            nc.gpsimd.memset(vt[:, :, :, W + 2 : W + 4], 0)
                            cs = slice(2, W + 2)
                            d0 = src[:, 0 : IR - 2 : 2, cs]
                            d1 = src[:, 1 : IR - 1 : 2, cs]
                            d2 = src[:, 2:IR:2, cs]
                            d3 = src[:, 3:IR:2, cs]
                            nc.vector.tensor_sub(vt[:, 0, :, cs], d0, d2)
                            nc.vector.tensor_add(vt[:, 1, :, cs], d1, d2)
                            nc.vector.tensor_sub(vt[:, 2, :, cs], d2, d1)
                            v3eng = nc.gpsimd if vt is vs else nc.vector
                            v3eng.tensor_sub(vt[:, 3, :, cs], d1, d3)
                        it += 1

                        for i in range(4):
                            for br, vt in ((0, vc), (1, vs)):
                                for dw in range(3):
                                    t_idx = ((i * 5 + j) * 2 + br) * 3 + dw
                                    nc.tensor.matmul(
                                        pss[i][:],
                                        wt[:, t_idx, :],
                                        vt[:, i, :, dw + 1 : dw + 65],
                                        start=(j == 0 and br == 0 and dw == 0),
                                        stop=(j == 4 and br == 1 and dw == 2),
                                    )

                        # previous chunk's inverse, deferred so it doesn't
                        # head-of-line-block this chunk's DVE/ACT gen work
                        if j == 2 and pending is not None:
                            emit_inverse(pending)
                            pending = None

                    pending = (b, r0, pss)
            if pending is not None:
                emit_inverse(pending)
    nc.finalize()
    return nc


def _get_module(reps=1, ht=HT):
    key = ("nc", reps, ht)
    if key not in _CACHE:
        _CACHE[key] = _build_module(reps, ht)
    return _CACHE[key]


def _host_weights(fc, mmdt=None):
    # fc: (2, O, C, kH, kW, G) -> winograd-H transform along kH, then pack
    # lhsT[p=(gp*64+c), t=((i*5+j)*2+br)*3+dw, o]
    import ml_dtypes

    Gw = np.array(
        [[1, 0, 0], [0.5, 0.5, 0.5], [0.5, -0.5, 0.5], [0, 0, 1]], np.float32
    )
    wt6 = np.einsum("iK,aocKVg->aociVg", Gw, fc)  # (2, O, C, 4, 3, G)
    wt7 = wt6.reshape(2, O, C, 4, 3, 5, 2)  # g -> (j, gp)
    # -> (gp, c, i, j, br, dw, o)
    wt8 = np.transpose(wt7, (6, 2, 3, 5, 0, 4, 1))
    return np.ascontiguousarray(
        wt8.reshape(128, NT, 128).astype(ml_dtypes.bfloat16)
    )


def _host_kvec():
    kvec = np.zeros((128, 5), np.float32)
    for j in range(5):
        kvec[0:64, j] = (2 * j + 1) / TWO_PI
        kvec[64:128, j] = (2 * j + 2) / TWO_PI
    return kvec


def kernel(x, fouriercoeffs, bias):
    x = np.ascontiguousarray(np.asarray(x, dtype=np.float32))
    fc = np.asarray(fouriercoeffs, dtype=np.float32)
    w_host = _host_weights(fc)
    kvec = _host_kvec()
    biasv = np.ascontiguousarray(
        np.asarray(bias, dtype=np.float32).reshape(128, 1)
    )

    nc = _get_module()
    in_maps = [
        {"x": x[i * BS : (i + 1) * BS], "w": w_host, "kvec": kvec, "biasv": biasv}
        for i in range(N_CORES)
    ]
    res = run_bass_kernel_spmd(nc, in_maps, list(range(N_CORES))).results
    return np.concatenate([res[i]["y"] for i in range(N_CORES)], axis=0)
